# revision 1
# baseline (speedup 1.0000x reference)
"""Lennard-Jones pair energies + per-atom segment sum on 8 Trainium2 cores.

Strategy (edge-partitioned, like GNN edge partitioning per the sharding hint):

Host (sharding step): pairs are partitioned by destination atom and packed
into a dense ELL-style layout -- atoms are grouped into sections by their
(padded) pair count L, each section laid out as [blocks, 128, L] so that one
SBUF partition holds one atom's L-slot pair run.  Pad slots use dist=RC, for
which the shifted LJ energy is exactly 0, so pads are numerically inert.
Atom sections are split evenly across the 8 cores (identical section shapes
per core, so one SPMD program serves all cores).

Device (all the arithmetic): for every pair slot computes the shifted LJ
energy and reduces each atom's L-run to its per-atom half-energy:
    en/2 = (v - b)^2 - 1/2 - e0/2,   v = sqrt(2)/d^6,  b = sqrt(1/2)
streamed as: w=d^2 (ACT), x=w^2 (ACT), y=(w*sqrt(.5))*x (DVE stt),
v=1/y (DVE reciprocal), bp=(v-b)^2 (ACT), grouped-sum over L (DVE reduce),
then a per-section constant fixup -L*(1/2+e0/2).

Host (unshard step): scatters the per-atom results back to atom order.
"""

import math

import numpy as np

RC = 3.0
N_CORES = 8
P = 128  # SBUF partitions
PAD_MULT = 4  # per-atom slot-count quantum
F_TARGET = 2048  # target free-dim elements per SBUF tile


def _build_layout(idx: np.ndarray, n_atoms: int):
    """Partition pairs by atom into dense per-core ELL sections.

    Returns (sections, slotmap, atom_of):
      sections: list of (L, m) with m blocks per core, identical across cores
      slotmap:  [N_CORES, W] pair id per slot (-1 = pad)
      atom_of:  [N_CORES, M_out, P] atom id per output cell (-1 = pad)
    """
    counts = np.bincount(idx, minlength=n_atoms)
    perm = np.argsort(idx, kind="stable")
    starts = np.zeros(n_atoms + 1, np.int64)
    starts[1:] = np.cumsum(counts)
    q = ((counts + PAD_MULT - 1) // PAD_MULT) * PAD_MULT
    has = counts > 0
    sections = []
    slot_chunks = [[] for _ in range(N_CORES)]
    atom_chunks = [[] for _ in range(N_CORES)]
    for L in np.unique(q[has]):
        atoms_L = np.where(has & (q == L))[0]
        n = len(atoms_L)
        n_pad = ((n + N_CORES * P - 1) // (N_CORES * P)) * (N_CORES * P)
        m = n_pad // (N_CORES * P)
        sections.append((int(L), int(m)))
        atoms_pad = np.full(n_pad, -1, np.int64)
        atoms_pad[:n] = atoms_L
        offs = np.arange(L)[None, :]
        cnt = np.where(atoms_pad >= 0, counts[np.maximum(atoms_pad, 0)], 0)
        valid = offs < cnt[:, None]
        src = starts[np.maximum(atoms_pad, 0)][:, None] + offs
        pairmat = np.full((n_pad, L), -1, np.int64)
        pairmat[valid] = perm[src[valid]]
        per_core = n_pad // N_CORES
        for c in range(N_CORES):
            chunk = pairmat[c * per_core : (c + 1) * per_core]
            slot_chunks[c].append(chunk.reshape(-1))
            atom_chunks[c].append(
                atoms_pad[c * per_core : (c + 1) * per_core].reshape(m, P)
            )
    slotmap = np.stack([np.concatenate(ch) for ch in slot_chunks])
    atom_of = np.stack([np.concatenate(ch, axis=0) for ch in atom_chunks])
    return sections, slotmap, atom_of


def _build_bass_program(sections, W: int):
    import concourse.bass as bass
    import concourse.tile as tile
    from concourse import bacc, mybir

    f32 = mybir.dt.float32
    e0 = 4.0 * ((1.0 / RC) ** 12 - (1.0 / RC) ** 6)
    beta = math.sqrt(0.5)
    m_total = sum(m for _, m in sections)

    nc = bacc.Bacc(
        "TRN2",
        target_bir_lowering=False,
        debug=False,
        enable_asserts=False,
        num_devices=N_CORES,
    )
    din = nc.dram_tensor("dist_packed", [W], f32, kind="ExternalInput")
    dout = nc.dram_tensor("en_blocks", [P, m_total], f32, kind="ExternalOutput")

    with tile.TileContext(nc) as tc:
        with (
            tc.tile_pool(name="io", bufs=6) as io_pool,
            tc.tile_pool(name="tmp", bufs=3) as tmp_pool,
            tc.tile_pool(name="acc", bufs=1) as acc_pool,
        ):
            out_raw = acc_pool.tile([P, m_total], f32, tag="out_raw")
            out_fin = acc_pool.tile([P, m_total], f32, tag="out_fin")
            nbias = acc_pool.tile([P, 1], f32, tag="nbias")
            nc.vector.memset(nbias[:], -beta)
            mscale = acc_pool.tile([P, 1], f32, tag="mscale")
            nc.vector.memset(mscale[:], -6.0)
            lbias = acc_pool.tile([P, 1], f32, tag="lbias")
            nc.vector.memset(lbias[:], math.log(math.sqrt(2.0)))
            off = 0
            col = 0
            for L, m in sections:
                sec = din.ap()[off : off + m * P * L].rearrange(
                    "(b p l) -> p b l", p=P, l=L
                )
                g_max = max(1, F_TARGET // L)
                b0 = 0
                while b0 < m:
                    g = min(g_max, m - b0)
                    F = g * L
                    d = io_pool.tile([P, F], f32, tag="d_in")
                    nc.sync.dma_start(
                        d[:].rearrange("p (b l) -> p b l", l=L),
                        sec[:, b0 : b0 + g, :],
                    )
                    # v = sqrt(2)/d^6 = exp(-6*ln(d) + ln(sqrt(2)))
                    t = tmp_pool.tile([P, F], f32, tag="t")
                    nc.scalar.activation(
                        t[:], d[:], mybir.ActivationFunctionType.Ln
                    )
                    v = tmp_pool.tile([P, F], f32, tag="v")
                    nc.scalar.activation(
                        v[:],
                        t[:],
                        mybir.ActivationFunctionType.Exp,
                        bias=lbias[:],
                        scale=mscale[:],
                    )
                    bp = tmp_pool.tile([P, F], f32, tag="bp")
                    nc.scalar.activation(
                        bp[:],
                        v[:],
                        mybir.ActivationFunctionType.Square,
                        bias=nbias[:],
                        scale=1.0,
                    )
                    nc.vector.tensor_reduce(
                        out_raw[:, col : col + g],
                        bp[:].rearrange("p (b l) -> p b l", l=L),
                        axis=mybir.AxisListType.X,
                        op=mybir.AluOpType.add,
                    )
                    b0 += g
                    col += g
                # per-section constant fixup: en/2 = sum(bp) - L*(1/2 + e0/2)
                nc.vector.tensor_scalar(
                    out_fin[:, col - m : col],
                    out_raw[:, col - m : col],
                    float(-L * (0.5 + e0 / 2.0)),
                    None,
                    mybir.AluOpType.add,
                )
                off += m * P * L
            nc.sync.dma_start(dout.ap(), out_fin[:])
    nc.compile()
    return nc


def kernel(**inputs) -> np.ndarray:
    dist = np.ascontiguousarray(np.asarray(inputs["dist"], dtype=np.float32))
    ind_2 = np.asarray(inputs["ind_2"])
    n_atoms = int(np.asarray(inputs["ind_1"]).shape[0])
    idx = ind_2[:, 0].astype(np.int64)

    sections, slotmap, atom_of = _build_layout(idx, n_atoms)
    W = slotmap.shape[1]

    in_maps = []
    for c in range(N_CORES):
        sm = slotmap[c]
        packed = np.where(sm >= 0, dist[np.maximum(sm, 0)], np.float32(RC))
        in_maps.append({"dist_packed": np.ascontiguousarray(packed, np.float32)})

    nc = _build_bass_program(sections, W)

    from concourse import bass_utils

    res = bass_utils.run_bass_kernel_spmd(
        nc, in_maps, core_ids=list(range(N_CORES))
    )

    out_full = np.zeros(n_atoms, np.float32)
    for c in range(N_CORES):
        dev = res.results[c]["en_blocks"]  # [P, M_out]
        a = atom_of[c]  # [M_out, P]
        valid = a >= 0
        out_full[a[valid]] = dev.T[valid]
    return out_full



# revision 7
# speedup vs baseline: 2.1583x; 2.1583x over previous
"""Lennard-Jones pair energies + per-atom segment sum on 8 Trainium2 cores.

Strategy (edge-partitioned per the sharding hint, ELL-style dense layout):

Host (sharding step): atoms are sorted by padded pair count and grouped into
chunks of 1024 (8 cores x 128 partitions); chunk i keeps L_i = max padded
count in the chunk, so there are no pad atoms and slot padding is minimal.
Each core receives a partition-major dense buffer [128, F_total] where
partition p's row is the concatenation over chunks of that atom's L_i-slot
run -- every DMA is fully contiguous per partition.  Pad slots use dist=RC
(shifted LJ energy exactly 0).  Slot 0 of each chunk is a host-computed
fixup distance d_L whose pair energy equals the column's additive constant
-L*e0/2, so the device-side reduce alone yields the final per-atom energy.

Device: one activation-table preload (ln/exp/square share a table set), then
per tile of ~1.1K columns: contiguous DMA, ACT ln, ACT exp (v = sqrt2*d^-6),
GPSIMD stt bp = (v - 2b)*v with b = sqrt(1/2)  (en/2 = bp - e0/2 per pair),
and DVE grouped tensor_reduce over each equal-L chunk run into the per-atom
output column.  One final DMA writes [128, n_chunks] back to HBM.

Host (unshard step): scatters per-atom results back to atom order.
"""

import math

import numpy as np

RC = 3.0
N_CORES = 8
P = 128
CH = N_CORES * P  # atoms per chunk
PAD_MULT = 4  # per-atom slot-count quantum
F_TARGET = 1152  # target free-dim columns per device tile

_E0 = 4.0 * ((1.0 / RC) ** 12 - (1.0 / RC) ** 6)
_B = math.sqrt(0.5)


def _build_layout(idx: np.ndarray, n_atoms: int, dist: np.ndarray):
    """Pack pairs into per-core partition-major ELL chunks.

    Returns (packed, atom_of, Lp, n_chunks):
      packed:  [N_CORES, P, F_total] f32 device input
      atom_of: [n_chunks, N_CORES, P] atom id per output cell (-1 = pad)
      Lp:      per-chunk padded width incl. fixup slot
    """
    counts = np.bincount(idx, minlength=n_atoms).astype(np.int64)
    perm = np.argsort(idx, kind="stable")
    starts = np.zeros(n_atoms + 1, np.int64)
    starts[1:] = np.cumsum(counts)
    q = ((counts + PAD_MULT - 1) // PAD_MULT) * PAD_MULT
    order = np.argsort(-q, kind="stable")
    n_chunks = (n_atoms + CH - 1) // CH
    n_pad = n_chunks * CH
    order_pad = np.full(n_pad, -1, np.int64)
    order_pad[:n_atoms] = order
    qs = np.where(order_pad >= 0, q[np.maximum(order_pad, 0)], 0)
    Lc = np.maximum(qs.reshape(n_chunks, CH).max(axis=1), PAD_MULT)
    Lp = Lc + 1  # fixup slot
    col0 = np.zeros(n_chunks + 1, np.int64)
    col0[1:] = np.cumsum(Lp)
    F_total = int(col0[-1])

    # fixup distance per chunk: bp(d_L) = -Lc*e0/2
    vfix = _B + np.sqrt(0.5 - Lc * _E0 / 2.0)
    dfix = (math.sqrt(2.0) / vfix) ** (1.0 / 6.0)

    dist_sorted = dist[perm]
    packed = np.full((N_CORES, P, F_total), np.float32(RC), np.float32)
    offs_max = np.arange(int(Lc.max()))
    for i in range(n_chunks):
        a = order_pad[i * CH : (i + 1) * CH]
        L = int(Lc[i])
        o = int(col0[i])
        cnt = np.where(a >= 0, counts[np.maximum(a, 0)], 0)
        offs = offs_max[:L][None, :]
        valid = offs < cnt[:, None]
        src = starts[np.maximum(a, 0)][:, None] + offs
        block = np.full((CH, L), np.float32(RC), np.float32)
        block[valid] = dist_sorted[src[valid]]
        packed[:, :, o + 1 : o + 1 + L] = block.reshape(N_CORES, P, L)
        packed[:, :, o] = np.float32(dfix[i])
    atom_of = order_pad.reshape(n_chunks, N_CORES, P)
    return packed, atom_of, [int(x) for x in Lp], n_chunks


def _tile_plan(Lp):
    """Group chunks into device tiles of ~F_TARGET columns.

    Returns list of tiles; each tile is (col_start, F, runs) with
    runs = [(tile_col_off, L, m, out_col)] for maximal equal-L chunk runs.
    """
    n = len(Lp)
    tiles = []
    c0 = 0
    width = 0
    bounds = []
    for i in range(n):
        if width + Lp[i] > F_TARGET and width > 0:
            bounds.append((c0, i))
            c0 = i
            width = 0
        width += Lp[i]
    bounds.append((c0, n))
    col = 0
    for c0, c1 in bounds:
        runs = []
        off = 0
        j = c0
        while j < c1:
            k = j
            while k < c1 and Lp[k] == Lp[j]:
                k += 1
            runs.append((off, Lp[j], k - j, j))
            off += Lp[j] * (k - j)
            j = k
        tiles.append((col, off, runs))
        col += off
    return tiles


def _build_bass_program(Lp, F_total, n_chunks):
    import concourse.bass as bass
    import concourse.tile as tile
    from concourse import bacc, mybir

    f32 = mybir.dt.float32
    bf16 = mybir.dt.bfloat16
    AF = mybir.ActivationFunctionType
    OP = mybir.AluOpType

    nc = bacc.Bacc(
        "TRN2",
        target_bir_lowering=False,
        debug=False,
        enable_asserts=False,
        num_devices=N_CORES,
    )
    din = nc.dram_tensor("dist_packed", [P, F_total], f32, kind="ExternalInput")
    dout = nc.dram_tensor("en_out", [P, n_chunks], f32, kind="ExternalOutput")

    # activation table set holding ln+exp together (one load for the whole
    # program instead of a 1.3us reload per function switch)
    set_id = 6
    try:
        from concourse.hw_specs import get_activation_tables

        for i, (_, funcs) in enumerate(get_activation_tables("TRN2").items()):
            if AF.Ln in funcs and AF.Exp in funcs:
                set_id = i
                break
    except Exception:
        pass

    tiles = _tile_plan(Lp)
    ln_sqrt2 = 0.5 * math.log(2.0)

    with tile.TileContext(nc) as tc:
        with (
            tc.tile_pool(name="io", bufs=5) as io_pool,
            tc.tile_pool(name="v", bufs=3) as vpool,
            tc.tile_pool(name="acc", bufs=1) as acc_pool,
        ):
            atl = mybir.InstLoadActFuncSet(
                name=nc.get_next_instruction_name(),
                ins=[],
                outs=[],
                act_func_set_id=set_id,
            )
            nc.scalar.add_instruction(atl)
            out_raw = acc_pool.tile([P, n_chunks], f32, tag="out_raw")
            lbias = acc_pool.tile([P, 1], f32, tag="lbias")
            nc.vector.memset(lbias[:], ln_sqrt2)
            for ti, (col, F, runs) in enumerate(tiles):
                d = io_pool.tile([P, F], f32, tag="d")
                half = (F // 2) & ~3
                nc.sync.dma_start(d[:, :half], din.ap()[:, col : col + half])
                nc.sync.dma_start(d[:, half:], din.ap()[:, col + half : col + F])
                nc.scalar.activation(d[:], d[:], AF.Ln)
                # v = sqrt2*d^-6 in bf16: halves DVE cycle cost (2x/4x
                # perf modes) for the stt below; l2 impact ~3e-3
                v = vpool.tile([P, F], bf16, tag="v")
                nc.scalar.activation(
                    v[:], d[:], AF.Exp, bias=lbias[:], scale=-6.0
                )
                # bp = (v - 2b)*v ; en/2 = bp - e0/2 (constant folded into
                # the per-chunk fixup slot)
                nc.vector.scalar_tensor_tensor(
                    v[:], v[:], 2.0 * _B, v[:], OP.subtract, OP.mult
                )
                for off, L, m, out_col in runs:
                    nc.vector.tensor_reduce(
                        out_raw[:, out_col : out_col + m],
                        v[:, off : off + m * L].rearrange(
                            "p (b l) -> p b l", l=L
                        ),
                        axis=mybir.AxisListType.X,
                        op=OP.add,
                    )
            nc.sync.dma_start(dout.ap(), out_raw[:])
    nc.compile()
    return nc


def _prepare(inputs):
    dist = np.ascontiguousarray(np.asarray(inputs["dist"], dtype=np.float32))
    ind_2 = np.asarray(inputs["ind_2"])
    n_atoms = int(np.asarray(inputs["ind_1"]).shape[0])
    idx = ind_2[:, 0].astype(np.int64)

    packed, atom_of, Lp, n_chunks = _build_layout(idx, n_atoms, dist)
    F_total = packed.shape[2]
    in_maps = [
        {"dist_packed": np.ascontiguousarray(packed[c])} for c in range(N_CORES)
    ]
    nc = _build_bass_program(Lp, F_total, n_chunks)
    return nc, in_maps, (atom_of, n_atoms)


def _finish(res, meta):
    atom_of, n_atoms = meta
    out_full = np.zeros(n_atoms, np.float32)
    for c in range(N_CORES):
        dev = res.results[c]["en_out"]  # [P, n_chunks]
        a = atom_of[:, c, :]  # [n_chunks, P]
        valid = a >= 0
        out_full[a[valid]] = dev.T[valid]
    return out_full


def kernel(**inputs) -> np.ndarray:
    nc, in_maps, meta = _prepare(inputs)

    from concourse import bass_utils

    res = bass_utils.run_bass_kernel_spmd(
        nc, in_maps, core_ids=list(range(N_CORES))
    )
    return _finish(res, meta)


# revision 13
# speedup vs baseline: 2.3620x; 1.0943x over previous
"""Lennard-Jones pair energies + per-atom segment sum on 8 Trainium2 cores.

Strategy (edge-partitioned per the sharding hint, ELL-style dense layout):

Host (sharding step): atoms are sorted by padded pair count and grouped into
chunks of 1024 (8 cores x 128 partitions); chunk i keeps L_i = max padded
count in the chunk, so there are no pad atoms and slot padding is minimal.
Each core receives a partition-major dense buffer [128, F_total] where
partition p's row is the concatenation over chunks of that atom's L_i-slot
run -- every DMA is fully contiguous per partition.  Pad slots use dist=RC
(shifted LJ energy exactly 0).  Slot 0 of each chunk is a host-computed
fixup distance d_L whose pair energy equals the column's additive constant
-L*e0/2, so the device-side reduce alone yields the final per-atom energy.

Device: one activation-table preload (ln/exp/square share a table set), then
per tile of ~1.1K columns: contiguous DMA, ACT ln, ACT exp (v = sqrt2*d^-6),
GPSIMD stt bp = (v - 2b)*v with b = sqrt(1/2)  (en/2 = bp - e0/2 per pair),
and DVE grouped tensor_reduce over each equal-L chunk run into the per-atom
output column.  One final DMA writes [128, n_chunks] back to HBM.

Host (unshard step): scatters per-atom results back to atom order.
"""

import math

import numpy as np

RC = 3.0
N_CORES = 8
P = 128
CH = N_CORES * P  # atoms per chunk
PAD_MULT = 4  # per-atom slot-count quantum
F_TARGET = 1152  # target free-dim columns per device tile

_E0 = 4.0 * ((1.0 / RC) ** 12 - (1.0 / RC) ** 6)
_B = math.sqrt(0.5)


def _merge_runs(Lc: np.ndarray, max_runs: int = 7, max_cost: int = 60000):
    """Round some chunks' L up to the next-larger run's L to cut the number
    of distinct L values. Lc is non-increasing (sorted desc)."""
    Lc = Lc.copy()
    while True:
        uniq = sorted(set(int(x) for x in Lc), reverse=True)
        if len(uniq) <= max_runs:
            break
        best = None
        for i in range(1, len(uniq)):
            src = uniq[i]
            dst = uniq[i - 1]
            m = int(np.sum(Lc == src))
            cost = m * CH * (dst - src)
            if best is None or cost < best[0]:
                best = (cost, src, dst)
        if best[0] > max_cost:
            break
        Lc[Lc == best[1]] = best[2]
    return Lc


def _build_layout(idx: np.ndarray, n_atoms: int, dist: np.ndarray):
    """Pack pairs into per-core partition-major ELL chunks.

    Returns (packed, atom_of, Lp, n_chunks):
      packed:  [N_CORES, P, F_total] f32 device input
      atom_of: [n_chunks, N_CORES, P] atom id per output cell (-1 = pad)
      Lp:      per-chunk padded width incl. fixup slot
    """
    counts = np.bincount(idx, minlength=n_atoms).astype(np.int64)
    perm = np.argsort(idx, kind="stable")
    starts = np.zeros(n_atoms + 1, np.int64)
    starts[1:] = np.cumsum(counts)
    q = ((counts + PAD_MULT - 1) // PAD_MULT) * PAD_MULT
    order = np.argsort(-q, kind="stable")
    n_chunks = (n_atoms + CH - 1) // CH
    n_pad = n_chunks * CH
    order_pad = np.full(n_pad, -1, np.int64)
    order_pad[:n_atoms] = order
    qs = np.where(order_pad >= 0, q[np.maximum(order_pad, 0)], 0)
    Lc = np.maximum(qs.reshape(n_chunks, CH).max(axis=1), PAD_MULT)
    # merge small equal-L runs upward (fewer distinct L values -> fewer
    # device reduce instructions) while the slot-padding cost stays tiny
    Lc = _merge_runs(Lc)
    Lp = Lc + 1  # fixup slot
    col0 = np.zeros(n_chunks + 1, np.int64)
    col0[1:] = np.cumsum(Lp)
    F_total = int(col0[-1])

    # fixup distance per chunk: bp(d_L) = -Lc*e0/2
    vfix = _B + np.sqrt(0.5 - Lc * _E0 / 2.0)
    dfix = (math.sqrt(2.0) / vfix) ** (1.0 / 6.0)

    dist_sorted = dist[perm]
    packed = np.full((N_CORES, P, F_total), np.float32(RC), np.float32)
    offs_max = np.arange(int(Lc.max()))
    for i in range(n_chunks):
        a = order_pad[i * CH : (i + 1) * CH]
        L = int(Lc[i])
        o = int(col0[i])
        cnt = np.where(a >= 0, counts[np.maximum(a, 0)], 0)
        offs = offs_max[:L][None, :]
        valid = offs < cnt[:, None]
        src = starts[np.maximum(a, 0)][:, None] + offs
        block = np.full((CH, L), np.float32(RC), np.float32)
        block[valid] = dist_sorted[src[valid]]
        packed[:, :, o + 1 : o + 1 + L] = block.reshape(N_CORES, P, L)
        packed[:, :, o] = np.float32(dfix[i])
    atom_of = order_pad.reshape(n_chunks, N_CORES, P)
    return packed, atom_of, [int(x) for x in Lp], n_chunks


def _tile_plan(Lp):
    """Group chunks into device tiles with a graded size schedule (small
    first tile for fast pipeline ramp, small last tile for a short tail).

    Returns list of tiles; each tile is (col_start, F, runs) with
    runs = [(tile_col_off, L, m, out_col)] for maximal equal-L chunk runs.
    """
    n = len(Lp)
    total = sum(Lp)
    # target cumulative boundaries as fractions of total width
    fracs = [0.08, 0.24, 0.46, 0.68, 0.90, 1.0]
    bounds = []
    c0 = 0
    width = 0
    col = 0
    fi = 0
    for i in range(n):
        width += Lp[i]
        col += Lp[i]
        if fi < len(fracs) - 1 and col >= fracs[fi] * total:
            bounds.append((c0, i + 1))
            c0 = i + 1
            width = 0
            fi += 1
    if c0 < n:
        bounds.append((c0, n))
    tiles = []
    col = 0
    for c0, c1 in bounds:
        runs = []
        off = 0
        j = c0
        while j < c1:
            k = j
            while k < c1 and Lp[k] == Lp[j]:
                k += 1
            runs.append((off, Lp[j], k - j, j))
            off += Lp[j] * (k - j)
            j = k
        tiles.append((col, off, runs))
        col += off
    return tiles


def _build_bass_program(Lp, F_total, n_chunks):
    import concourse.bass as bass
    import concourse.tile as tile
    from concourse import bacc, mybir

    f32 = mybir.dt.float32
    bf16 = mybir.dt.bfloat16
    AF = mybir.ActivationFunctionType
    OP = mybir.AluOpType

    nc = bacc.Bacc(
        "TRN2",
        target_bir_lowering=False,
        debug=False,
        enable_asserts=False,
        num_devices=N_CORES,
    )
    din = nc.dram_tensor("dist_packed", [P, F_total], f32, kind="ExternalInput")
    dout = nc.dram_tensor("en_out", [P, n_chunks], f32, kind="ExternalOutput")

    # activation table set holding ln+exp together (one load for the whole
    # program instead of a 1.3us reload per function switch)
    set_id = 6
    try:
        from concourse.hw_specs import get_activation_tables

        for i, (_, funcs) in enumerate(get_activation_tables("TRN2").items()):
            if AF.Ln in funcs and AF.Exp in funcs:
                set_id = i
                break
    except Exception:
        pass

    tiles = _tile_plan(Lp)
    ln_sqrt2 = 0.5 * math.log(2.0)

    with tile.TileContext(nc) as tc:
        with (
            tc.tile_pool(name="io", bufs=4) as io_pool,
            tc.tile_pool(name="v", bufs=3) as vpool,
            tc.tile_pool(name="u", bufs=2) as upool,
            tc.tile_pool(name="acc", bufs=1) as acc_pool,
        ):
            atl = mybir.InstLoadActFuncSet(
                name=nc.get_next_instruction_name(),
                ins=[],
                outs=[],
                act_func_set_id=set_id,
            )
            nc.scalar.add_instruction(atl)
            out_raw = acc_pool.tile([P, n_chunks], f32, tag="out_raw")
            lbias = acc_pool.tile([P, 1], f32, tag="lbias")
            nc.vector.memset(lbias[:], ln_sqrt2)
            for ti, (col, F, runs) in enumerate(tiles):
                d = io_pool.tile([P, F], f32, tag="d")
                nc.sync.dma_start(d[:], din.ap()[:, col : col + F])
                nc.scalar.activation(d[:], d[:], AF.Ln)
                # v = sqrt2*d^-6 in bf16: tensor_scalar runs 4x and
                # tensor_tensor 2x on 2-byte dtypes; l2 impact ~3e-3
                v = vpool.tile([P, F], bf16, tag="v")
                nc.scalar.activation(
                    v[:], d[:], AF.Exp, bias=lbias[:], scale=-6.0
                )
                # bp = (v - 2b)*v ; en/2 = bp - e0/2 (constant folded into
                # the per-chunk fixup slot).  Split as ts (4x) + tt (2x)
                # instead of scalar_tensor_tensor (1x on hardware).
                u = upool.tile([P, F], bf16, tag="u")
                nc.vector.tensor_scalar(u[:], v[:], 2.0 * _B, None, OP.subtract)
                nc.vector.tensor_tensor(v[:], u[:], v[:], OP.mult)
                c0 = runs[0][3]
                c1 = runs[-1][3] + runs[-1][2]
                for off, L, m, out_col in runs:
                    nc.vector.tensor_reduce(
                        out_raw[:, out_col : out_col + m],
                        v[:, off : off + m * L].rearrange(
                            "p (b l) -> p b l", l=L
                        ),
                        axis=mybir.AxisListType.X,
                        op=OP.add,
                    )
                nc.sync.dma_start(
                    dout.ap()[:, c0:c1], out_raw[:, c0:c1]
                )
    nc.compile()
    return nc


def _prepare(inputs):
    dist = np.ascontiguousarray(np.asarray(inputs["dist"], dtype=np.float32))
    ind_2 = np.asarray(inputs["ind_2"])
    n_atoms = int(np.asarray(inputs["ind_1"]).shape[0])
    idx = ind_2[:, 0].astype(np.int64)

    packed, atom_of, Lp, n_chunks = _build_layout(idx, n_atoms, dist)
    F_total = packed.shape[2]
    in_maps = [
        {"dist_packed": np.ascontiguousarray(packed[c])} for c in range(N_CORES)
    ]
    nc = _build_bass_program(Lp, F_total, n_chunks)
    return nc, in_maps, (atom_of, n_atoms)


def _finish(res, meta):
    atom_of, n_atoms = meta
    out_full = np.zeros(n_atoms, np.float32)
    for c in range(N_CORES):
        dev = res.results[c]["en_out"]  # [P, n_chunks]
        a = atom_of[:, c, :]  # [n_chunks, P]
        valid = a >= 0
        out_full[a[valid]] = dev.T[valid]
    return out_full


def kernel(**inputs) -> np.ndarray:
    nc, in_maps, meta = _prepare(inputs)

    from concourse import bass_utils

    res = bass_utils.run_bass_kernel_spmd(
        nc, in_maps, core_ids=list(range(N_CORES))
    )
    return _finish(res, meta)


# revision 18
# speedup vs baseline: 2.3838x; 1.0092x over previous
"""Lennard-Jones pair energies + per-atom segment sum on 8 Trainium2 cores.

Strategy (edge-partitioned per the sharding hint, ELL-style dense layout):

Host (sharding step): atoms are sorted by padded pair count and grouped into
chunks of 1024 (8 cores x 128 partitions); chunk i keeps L_i = max padded
count in the chunk, so there are no pad atoms and slot padding is minimal.
Each core receives a partition-major dense buffer [128, F_total] where
partition p's row is the concatenation over chunks of that atom's L_i-slot
run -- every DMA is fully contiguous per partition.  Pad slots use dist=RC
(shifted LJ energy exactly 0).  Slot 0 of each chunk is a host-computed
fixup distance d_L whose pair energy equals the column's additive constant
-L*e0/2, so the device-side reduce alone yields the final per-atom energy.

Device: one activation-table preload (ln/exp/square share a table set), then
per tile of ~1.1K columns: contiguous DMA, ACT ln, ACT exp (v = sqrt2*d^-6),
GPSIMD stt bp = (v - 2b)*v with b = sqrt(1/2)  (en/2 = bp - e0/2 per pair),
and DVE grouped tensor_reduce over each equal-L chunk run into the per-atom
output column.  One final DMA writes [128, n_chunks] back to HBM.

Host (unshard step): scatters per-atom results back to atom order.
"""

import math

import numpy as np

RC = 3.0
N_CORES = 8
P = 128
CH = N_CORES * P  # atoms per chunk
PAD_MULT = 2  # per-atom slot-count quantum

_E0 = 4.0 * ((1.0 / RC) ** 12 - (1.0 / RC) ** 6)
_B = math.sqrt(0.5)


def _merge_runs(Lc: np.ndarray, max_runs: int = 7, max_cost: int = 60000):
    """Round some chunks' L up to the next-larger run's L to cut the number
    of distinct L values. Lc is non-increasing (sorted desc)."""
    Lc = Lc.copy()
    while True:
        uniq = sorted(set(int(x) for x in Lc), reverse=True)
        if len(uniq) <= max_runs:
            break
        best = None
        for i in range(1, len(uniq)):
            src = uniq[i]
            dst = uniq[i - 1]
            m = int(np.sum(Lc == src))
            cost = m * CH * (dst - src)
            if best is None or cost < best[0]:
                best = (cost, src, dst)
        if best[0] > max_cost:
            break
        Lc[Lc == best[1]] = best[2]
    return Lc


def _build_layout(idx: np.ndarray, n_atoms: int, dist: np.ndarray):
    """Pack pairs into per-core partition-major ELL chunks.

    Returns (packed, atom_of, Lp, n_chunks):
      packed:  [N_CORES, P, F_total] f32 device input
      atom_of: [n_chunks, N_CORES, P] atom id per output cell (-1 = pad)
      Lp:      per-chunk padded width incl. fixup slot
    """
    counts = np.bincount(idx, minlength=n_atoms).astype(np.int64)
    perm = np.argsort(idx, kind="stable")
    starts = np.zeros(n_atoms + 1, np.int64)
    starts[1:] = np.cumsum(counts)
    q = ((counts + PAD_MULT - 1) // PAD_MULT) * PAD_MULT
    order = np.argsort(-q, kind="stable")
    n_chunks = (n_atoms + CH - 1) // CH
    n_pad = n_chunks * CH
    order_pad = np.full(n_pad, -1, np.int64)
    order_pad[:n_atoms] = order
    qs = np.where(order_pad >= 0, q[np.maximum(order_pad, 0)], 0)
    Lc = np.maximum(qs.reshape(n_chunks, CH).max(axis=1), PAD_MULT)
    # merge small equal-L runs upward (fewer distinct L values -> fewer
    # device reduce instructions) while the slot-padding cost stays tiny
    Lc = _merge_runs(Lc)
    Lp = Lc + 1  # fixup slot
    col0 = np.zeros(n_chunks + 1, np.int64)
    col0[1:] = np.cumsum(Lp)
    F_total = int(col0[-1])

    # fixup distance per chunk: bp(d_L) = -Lc*e0/2
    vfix = _B + np.sqrt(0.5 - Lc * _E0 / 2.0)
    dfix = (math.sqrt(2.0) / vfix) ** (1.0 / 6.0)

    dist_sorted = dist[perm].astype(np.float16)
    packed = np.full((N_CORES, P, F_total), np.float16(RC), np.float16)
    offs_max = np.arange(int(Lc.max()))
    for i in range(n_chunks):
        a = order_pad[i * CH : (i + 1) * CH]
        L = int(Lc[i])
        o = int(col0[i])
        cnt = np.where(a >= 0, counts[np.maximum(a, 0)], 0)
        offs = offs_max[:L][None, :]
        valid = offs < cnt[:, None]
        src = starts[np.maximum(a, 0)][:, None] + offs
        block = np.full((CH, L), np.float16(RC), np.float16)
        block[valid] = dist_sorted[src[valid]]
        packed[:, :, o + 1 : o + 1 + L] = block.reshape(N_CORES, P, L)
        packed[:, :, o] = np.float16(dfix[i])
    atom_of = order_pad.reshape(n_chunks, N_CORES, P)
    return packed, atom_of, [int(x) for x in Lp], n_chunks


def _tile_plan(Lp):
    """Group chunks into device tiles with a graded size schedule (small
    first tile for fast pipeline ramp, small last tile for a short tail).

    Returns list of tiles; each tile is (col_start, F, runs) with
    runs = [(tile_col_off, L, m, out_col)] for maximal equal-L chunk runs.
    """
    n = len(Lp)
    total = sum(Lp)
    # target cumulative boundaries as fractions of total width
    fracs = [0.08, 0.24, 0.46, 0.68, 0.90, 1.0]
    bounds = []
    c0 = 0
    width = 0
    col = 0
    fi = 0
    for i in range(n):
        width += Lp[i]
        col += Lp[i]
        if fi < len(fracs) - 1 and col >= fracs[fi] * total:
            bounds.append((c0, i + 1))
            c0 = i + 1
            width = 0
            fi += 1
    if c0 < n:
        bounds.append((c0, n))
    tiles = []
    col = 0
    for c0, c1 in bounds:
        runs = []
        off = 0
        j = c0
        while j < c1:
            k = j
            while k < c1 and Lp[k] == Lp[j]:
                k += 1
            runs.append((off, Lp[j], k - j, j))
            off += Lp[j] * (k - j)
            j = k
        tiles.append((col, off, runs))
        col += off
    return tiles


def _build_bass_program(Lp, F_total, n_chunks):
    import concourse.bass as bass
    import concourse.tile as tile
    from concourse import bacc, mybir

    f32 = mybir.dt.float32
    f16 = mybir.dt.float16
    AF = mybir.ActivationFunctionType
    OP = mybir.AluOpType

    nc = bacc.Bacc(
        "TRN2",
        target_bir_lowering=False,
        debug=False,
        enable_asserts=False,
        num_devices=N_CORES,
    )
    din = nc.dram_tensor("dist_packed", [P, F_total], f16, kind="ExternalInput")
    dout = nc.dram_tensor("en_out", [P, n_chunks], f32, kind="ExternalOutput")

    # activation table set holding ln+exp together (one load for the whole
    # program instead of a 1.3us reload per function switch)
    set_id = 6
    try:
        from concourse.hw_specs import get_activation_tables

        for i, (_, funcs) in enumerate(get_activation_tables("TRN2").items()):
            if AF.Ln in funcs and AF.Exp in funcs:
                set_id = i
                break
    except Exception:
        pass

    tiles = _tile_plan(Lp)
    ln_sqrt2 = 0.5 * math.log(2.0)

    with tile.TileContext(nc) as tc:
        with (
            tc.tile_pool(name="io", bufs=4) as io_pool,
            tc.tile_pool(name="t", bufs=2) as tpool,
            tc.tile_pool(name="u", bufs=2) as upool,
            tc.tile_pool(name="acc", bufs=1) as acc_pool,
        ):
            atl = mybir.InstLoadActFuncSet(
                name=nc.get_next_instruction_name(),
                ins=[],
                outs=[],
                act_func_set_id=set_id,
            )
            nc.scalar.add_instruction(atl)
            out_raw = acc_pool.tile([P, n_chunks], f32, tag="out_raw")
            lbias = acc_pool.tile([P, 1], f32, tag="lbias")
            nc.vector.memset(lbias[:], ln_sqrt2)
            for ti, (col, F, runs) in enumerate(tiles):
                d = io_pool.tile([P, F], f16, tag="d")
                nc.sync.dma_start(d[:], din.ap()[:, col : col + F])
                # t = ln(d) at f32 (exp amplifies ln error 6x)
                t = tpool.tile([P, F], f32, tag="t")
                nc.scalar.activation(t[:], d[:], AF.Ln)
                # v = sqrt2*d^-6 in fp16: tensor_scalar runs 4x and
                # tensor_tensor 2x on 2-byte dtypes; overall l2 ~1.6e-3
                nc.scalar.activation(
                    d[:], t[:], AF.Exp, bias=lbias[:], scale=-6.0
                )
                # bp = (v - 2b)*v ; en/2 = bp - e0/2 (constant folded into
                # the per-chunk fixup slot).  Split as ts (4x) + tt (2x)
                # instead of scalar_tensor_tensor (1x on hardware).
                v = d
                u = upool.tile([P, F], f16, tag="u")
                nc.vector.tensor_scalar(u[:], v[:], 2.0 * _B, None, OP.subtract)
                nc.vector.tensor_tensor(v[:], u[:], v[:], OP.mult)
                c0 = runs[0][3]
                c1 = runs[-1][3] + runs[-1][2]
                for off, L, m, out_col in runs:
                    nc.vector.tensor_reduce(
                        out_raw[:, out_col : out_col + m],
                        v[:, off : off + m * L].rearrange(
                            "p (b l) -> p b l", l=L
                        ),
                        axis=mybir.AxisListType.X,
                        op=OP.add,
                    )
                nc.sync.dma_start(
                    dout.ap()[:, c0:c1], out_raw[:, c0:c1]
                )
    nc.compile()
    return nc


def _prepare(inputs):
    dist = np.ascontiguousarray(np.asarray(inputs["dist"], dtype=np.float32))
    ind_2 = np.asarray(inputs["ind_2"])
    n_atoms = int(np.asarray(inputs["ind_1"]).shape[0])
    idx = ind_2[:, 0].astype(np.int64)

    packed, atom_of, Lp, n_chunks = _build_layout(idx, n_atoms, dist)
    F_total = packed.shape[2]
    in_maps = [
        {"dist_packed": np.ascontiguousarray(packed[c])} for c in range(N_CORES)
    ]
    nc = _build_bass_program(Lp, F_total, n_chunks)
    return nc, in_maps, (atom_of, n_atoms)


def _finish(res, meta):
    atom_of, n_atoms = meta
    out_full = np.zeros(n_atoms, np.float32)
    for c in range(N_CORES):
        dev = res.results[c]["en_out"]  # [P, n_chunks]
        a = atom_of[:, c, :]  # [n_chunks, P]
        valid = a >= 0
        out_full[a[valid]] = dev.T[valid]
    return out_full


def kernel(**inputs) -> np.ndarray:
    nc, in_maps, meta = _prepare(inputs)

    from concourse import bass_utils

    res = bass_utils.run_bass_kernel_spmd(
        nc, in_maps, core_ids=list(range(N_CORES))
    )
    return _finish(res, meta)
